# revision 4
# baseline (speedup 1.0000x reference)
"""2D Haar DWT (periodization) on Trainium2, data-parallel over 8 NeuronCores.

fp16-in / int8-out, stage-H on the PE (tensor) engine. The int8-out
kernel is DVE-bound (73us butterfly vs ~63us DMA). Here the host
pre-permutes input rows so each H-pair lands on an adjacent partition
pair (2q, 2q+1); stage-H then becomes ONE matmul per 512-column chunk
against a stationary 128x128 block-diagonal butterfly matrix
[[1,1],[1,-1]] — computed by the otherwise-idle PE into PSUM with exact
fp32 accumulation. The fp32->int8 casting copies are split between the
Activation engine and the DVE (which now only does stage W). All DMAs
stay plain contiguous; the host decodes the resulting layout.

Engine budget/core (clean): DMA ~63us, PE ~45, DVE ~53, ACT ~50.
"""

import sys

import numpy as np

if "/opt/trn_rl_repo" not in sys.path:
    sys.path.insert(0, "/opt/trn_rl_repo")

B, C, H, W = 8, 32, 512, 512
ROWS = C * H              # 16384 flat rows per core (each [e256|o256] fp16)
RPP = 16                  # input rows per partition (must be even)
TILE_ROWS = 128 * RPP     # 2048
OROWS = ROWS // 2         # 8192 fused output rows per core
N_CORES = 8

TAPER = 4
PLAN = [(i * TILE_ROWS, TILE_ROWS) for i in range(ROWS // TILE_ROWS - 1)]
PLAN += [
    ((ROWS - TILE_ROWS) + k * (TILE_ROWS // TAPER), TILE_ROWS // TAPER)
    for k in range(TAPER)
]

_cache = {}


def _perm():
    """src row for each device row: device partition p, block j of tile
    (r0,nrows) reads original row 2*(r0/2 + j*64 + p//2) + (p&1)."""
    src = np.empty(ROWS, np.int64)
    for r0, nrows in PLAN:
        rpp = nrows // 128
        p = np.arange(128)[:, None]
        j = np.arange(rpp)[None, :]
        dest = r0 + p * rpp + j
        srcr = 2 * (r0 // 2 + j * 64 + p // 2) + (p & 1)
        src[dest.ravel()] = srcr.ravel()
    return src


def _bmat():
    bm = np.zeros((128, 128), np.float16)
    q = np.arange(64)
    bm[2 * q, 2 * q] = 1
    bm[2 * q + 1, 2 * q] = 1
    bm[2 * q, 2 * q + 1] = 1
    bm[2 * q + 1, 2 * q + 1] = -1
    return bm


def _build_program():
    from concourse import bacc, mybir
    from concourse.tile import TileContext

    f16 = mybir.dt.float16
    f32 = mybir.dt.float32
    i8 = mybir.dt.int8

    nc = bacc.Bacc()
    x = nc.dram_tensor("x", [ROWS, W], f16, kind="ExternalInput")
    bm = nc.dram_tensor("bm", [128, 128], f16, kind="ExternalInput")
    y = nc.dram_tensor("y", [OROWS, 2 * W], i8, kind="ExternalOutput")

    with TileContext(nc) as tc, \
         tc.tile_pool(name="pb", bufs=1) as pb, \
         tc.tile_pool(name="p", bufs=3) as pool, \
         tc.tile_pool(name="pp", bufs=6, space="PSUM") as pps:
        bt = pb.tile([128, 128], f16, tag="bmat")
        nc.sync.dma_start(bt[:], bm[:, :])

        for r0, nrows in PLAN:
            rpp = nrows // 128        # rows per partition this tile
            nch = rpp // 2            # 512-col matmul chunks per half

            tin = pool.tile([128, rpp * W], f16, tag="tin",
                            padded_shape=[128, RPP * W])
            nc.sync.dma_start(tin[:], x[r0 : r0 + nrows, :])

            t4 = tin.rearrange("p (j s w) -> p j s w", j=rpp, s=2)
            e = t4[:, :, 0, :]
            o = t4[:, :, 1, :]

            s = pool.tile([128, rpp * (W // 2)], f16, tag="s",
                          padded_shape=[128, RPP * (W // 2)])
            d = pool.tile([128, rpp * (W // 2)], f16, tag="d",
                          padded_shape=[128, RPP * (W // 2)])
            s3 = s.rearrange("p (j w) -> p j w", j=rpp)
            d3 = d.rearrange("p (j w) -> p j w", j=rpp)
            nc.vector.tensor_add(out=s3, in0=e, in1=o)
            nc.vector.tensor_sub(out=d3, in0=e, in1=o)

            t8 = pool.tile([128, rpp * W], i8, tag="t8",
                           padded_shape=[128, RPP * W])

            ncast = 2 * nch
            dve_casts = max(1, (3 * ncast) // 16)   # ~3/16 of casts on DVE
            ci = 0
            for src in (s, d):
                for c in range(nch):
                    ps = pps.tile([128, 512], f32, tag="ps")
                    nc.tensor.matmul(
                        ps[:], bt[:], src[:, c * 512 : (c + 1) * 512],
                        start=True, stop=True,
                    )
                    sl = t8[:, ci * 512 : (ci + 1) * 512]
                    # spread DVE casts evenly through the chunk sequence
                    if (ci * dve_casts) % ncast >= ncast - dve_casts:
                        nc.vector.tensor_copy(out=sl, in_=ps[:])
                    else:
                        nc.scalar.copy(out=sl, in_=ps[:])
                    ci += 1

            orow = r0 // 2
            nc.sync.dma_start(y[orow : orow + nrows // 2, :], t8[:])

    nc.finalize()
    return nc


def _run(x, trace=False):
    from concourse.bass_utils import run_bass_kernel_spmd

    if "nc" not in _cache:
        _cache["nc"] = _build_program()
    nc = _cache["nc"]

    x = np.asarray(x)
    mx = float(np.abs(x).max())
    sc = 2.0 * mx / 127.0         # |subband| <= 2*max|x| -> no saturation
    alpha = np.float32(0.5 / sc)  # folds the DWT's 0.5 and the 1/s

    xh = np.empty((B, C, H, W), np.float16)
    xh[..., : W // 2] = x[..., 0::2] * alpha
    xh[..., W // 2 :] = x[..., 1::2] * alpha
    xh = xh.reshape(B, ROWS, W)
    xdev = np.ascontiguousarray(xh[:, _perm(), :])

    bmat = _bmat()
    in_maps = [{"x": xdev[i], "bm": bmat} for i in range(N_CORES)]
    res = run_bass_kernel_spmd(nc, in_maps, core_ids=list(range(N_CORES)), trace=trace)
    _cache["last_results"] = res

    ys = np.stack([res.results[i]["y"] for i in range(N_CORES)])
    subs = _decode(ys)
    sf = np.float32(sc)
    return tuple(a.astype(np.float32) * sf for a in subs)


def _decode(ys):
    """ys: [B, 8192, 1024] int8 -> (LL, LH, HL, HH) int8 [B, C, 256, 256].

    Device y rows [r0/2, r0/2+nrows/2) of a tile hold t8 [128, rpp*512]:
    t8[p, (half*nch + c)*512 + u] = subband(half, p&1)[k, w] with
    j = 2c + u//256, w = u%256, k = r0/2 + j*64 + p//2.
    """
    out = np.empty((4, ys.shape[0], OROWS, W // 2), np.int8)
    for r0, nrows in PLAN:
        rpp = nrows // 128
        nch = rpp // 2
        blk = ys[:, r0 // 2 : (r0 + nrows) // 2, :]
        blk = blk.reshape(ys.shape[0], 128, 2, nch, 2, W // 2)
        # axes: [B, p, half, c, jparity(u//256), w]
        p = np.arange(128)
        for half in range(2):
            for hh in range(2):          # p & 1
                sub = 2 * half + hh
                rows = blk[:, p[p % 2 == hh], half]   # [B, 64, nch, 2, 256]
                # k = r0/2 + (2c + jp)*64 + q  for q in [0,64)
                for c in range(nch):
                    for jp in range(2):
                        k0 = r0 // 2 + (2 * c + jp) * 64
                        out[sub, :, k0 : k0 + 64, :] = rows[:, :, c, jp, :]
    return [out[i].reshape(ys.shape[0], C, H // 2, W // 2) for i in range(4)]


def kernel(x):
    return _run(x, trace=False)


# revision 5
# speedup vs baseline: 1.0577x; 1.0577x over previous
"""2D Haar DWT (periodization) on Trainium2, data-parallel over 8 NeuronCores.

fp16-in / int8-out, stage-H on the PE (tensor) engine. The int8-out
kernel is DVE-bound (73us butterfly vs ~63us DMA). Here the host
pre-permutes input rows so each H-pair lands on an adjacent partition
pair (2q, 2q+1); stage-H then becomes ONE matmul per 512-column chunk
against a stationary 128x128 block-diagonal butterfly matrix
[[1,1],[1,-1]] — computed by the otherwise-idle PE into PSUM with exact
fp32 accumulation. The fp32->int8 casting copies are split between the
Activation engine and the DVE (which now only does stage W). All DMAs
stay plain contiguous; the host decodes the resulting layout.

Engine budget/core (clean): DMA ~63us, PE ~45, DVE ~53, ACT ~50.
"""

import sys

import numpy as np

if "/opt/trn_rl_repo" not in sys.path:
    sys.path.insert(0, "/opt/trn_rl_repo")

B, C, H, W = 8, 32, 512, 512
ROWS = C * H              # 16384 flat rows per core (each [e256|o256] fp16)
RPP = 16                  # input rows per partition (must be even)
TILE_ROWS = 128 * RPP     # 2048
OROWS = ROWS // 2         # 8192 fused output rows per core
N_CORES = 8

TAPER = 4
PLAN = [(i * TILE_ROWS, TILE_ROWS) for i in range(ROWS // TILE_ROWS - 1)]
PLAN += [
    ((ROWS - TILE_ROWS) + k * (TILE_ROWS // TAPER), TILE_ROWS // TAPER)
    for k in range(TAPER)
]

_cache = {}


def _perm():
    """src row for each device row: device partition p, block j of tile
    (r0,nrows) reads original row 2*(r0/2 + j*64 + p//2) + (p&1)."""
    src = np.empty(ROWS, np.int64)
    for r0, nrows in PLAN:
        rpp = nrows // 128
        p = np.arange(128)[:, None]
        j = np.arange(rpp)[None, :]
        dest = r0 + p * rpp + j
        srcr = 2 * (r0 // 2 + j * 64 + p // 2) + (p & 1)
        src[dest.ravel()] = srcr.ravel()
    return src


def _bmat():
    bm = np.zeros((128, 128), np.float16)
    q = np.arange(64)
    bm[2 * q, 2 * q] = 1
    bm[2 * q + 1, 2 * q] = 1
    bm[2 * q, 2 * q + 1] = 1
    bm[2 * q + 1, 2 * q + 1] = -1
    return bm


def _build_program():
    from concourse import bacc, mybir
    from concourse.tile import TileContext

    f16 = mybir.dt.float16
    f32 = mybir.dt.float32
    i8 = mybir.dt.int8

    nc = bacc.Bacc()
    x = nc.dram_tensor("x", [ROWS, W], f16, kind="ExternalInput")
    bm = nc.dram_tensor("bm", [128, 128], f16, kind="ExternalInput")
    y = nc.dram_tensor("y", [OROWS, 2 * W], i8, kind="ExternalOutput")

    with TileContext(nc) as tc, \
         tc.tile_pool(name="pb", bufs=1) as pb, \
         tc.tile_pool(name="p", bufs=4) as pool, \
         tc.tile_pool(name="pp", bufs=8, space="PSUM") as pps:
        bt = pb.tile([128, 128], f16, tag="bmat")
        nc.sync.dma_start(bt[:], bm[:, :])

        for r0, nrows in PLAN:
            rpp = nrows // 128        # rows per partition this tile
            nch = rpp // 2            # 512-col matmul chunks per half

            tin = pool.tile([128, rpp * W], f16, tag="tin",
                            padded_shape=[128, RPP * W])
            nc.sync.dma_start(tin[:], x[r0 : r0 + nrows, :])

            t4 = tin.rearrange("p (j s w) -> p j s w", j=rpp, s=2)
            e = t4[:, :, 0, :]
            o = t4[:, :, 1, :]

            s = pool.tile([128, rpp * (W // 2)], f16, tag="s",
                          padded_shape=[128, RPP * (W // 2)])
            d = pool.tile([128, rpp * (W // 2)], f16, tag="d",
                          padded_shape=[128, RPP * (W // 2)])
            s3 = s.rearrange("p (j w) -> p j w", j=rpp)
            d3 = d.rearrange("p (j w) -> p j w", j=rpp)
            nc.vector.tensor_add(out=s3, in0=e, in1=o)
            nc.vector.tensor_sub(out=d3, in0=e, in1=o)

            t8 = pool.tile([128, rpp * W], i8, tag="t8",
                           padded_shape=[128, RPP * W])

            ncast = 2 * nch
            dve_casts = max(1, ncast // 4)          # ~4/16 of casts on DVE
            ci = 0
            for src in (s, d):
                for c in range(nch):
                    ps = pps.tile([128, 512], f32, tag="ps")
                    nc.tensor.matmul(
                        ps[:], bt[:], src[:, c * 512 : (c + 1) * 512],
                        start=True, stop=True,
                    )
                    sl = t8[:, ci * 512 : (ci + 1) * 512]
                    # spread DVE casts evenly through the chunk sequence
                    if (ci * dve_casts) % ncast >= ncast - dve_casts:
                        nc.vector.tensor_copy(out=sl, in_=ps[:])
                    else:
                        nc.scalar.copy(out=sl, in_=ps[:])
                    ci += 1

            # drain each half of the output as soon as its casts finish
            orow = r0 // 2
            hrows = nrows // 4
            nc.sync.dma_start(y[orow : orow + hrows, :],
                              t8[:, : rpp * W // 2])
            nc.sync.dma_start(y[orow + hrows : orow + 2 * hrows, :],
                              t8[:, rpp * W // 2 :])

    nc.finalize()
    return nc


def _run(x, trace=False):
    from concourse.bass_utils import run_bass_kernel_spmd

    if "nc" not in _cache:
        _cache["nc"] = _build_program()
    nc = _cache["nc"]

    x = np.asarray(x)
    mx = float(np.abs(x).max())
    sc = 2.0 * mx / 127.0         # |subband| <= 2*max|x| -> no saturation
    alpha = np.float32(0.5 / sc)  # folds the DWT's 0.5 and the 1/s

    xh = np.empty((B, C, H, W), np.float16)
    xh[..., : W // 2] = x[..., 0::2] * alpha
    xh[..., W // 2 :] = x[..., 1::2] * alpha
    xh = xh.reshape(B, ROWS, W)
    xdev = np.ascontiguousarray(xh[:, _perm(), :])

    bmat = _bmat()
    in_maps = [{"x": xdev[i], "bm": bmat} for i in range(N_CORES)]
    res = run_bass_kernel_spmd(nc, in_maps, core_ids=list(range(N_CORES)), trace=trace)
    _cache["last_results"] = res

    ys = np.stack([res.results[i]["y"] for i in range(N_CORES)])
    subs = _decode(ys)
    sf = np.float32(sc)
    return tuple(a.astype(np.float32) * sf for a in subs)


def _decode(ys):
    """ys: [B, 8192, 1024] int8 -> (LL, LH, HL, HH) int8 [B, C, 256, 256].

    Device y rows [r0/2, r0/2+nrows/2) of a tile hold t8 [128, rpp*512]:
    t8[p, (half*nch + c)*512 + u] = subband(half, p&1)[k, w] with
    j = 2c + u//256, w = u%256, k = r0/2 + j*64 + p//2.
    """
    out = np.empty((4, ys.shape[0], OROWS, W // 2), np.int8)
    for r0, nrows in PLAN:
        rpp = nrows // 128
        nch = rpp // 2
        blk = ys[:, r0 // 2 : (r0 + nrows) // 2, :]
        # split out-DMAs: row block = [s-half rows | d-half rows]
        blk = blk.reshape(ys.shape[0], 2, 128, nch, 2, W // 2)
        # axes: [B, half, p, c, jparity(u//256), w]
        p = np.arange(128)
        for half in range(2):
            for hh in range(2):          # p & 1
                sub = 2 * half + hh
                rows = blk[:, half, p[p % 2 == hh]]   # [B, 64, nch, 2, 256]
                # k = r0/2 + (2c + jp)*64 + q  for q in [0,64)
                for c in range(nch):
                    for jp in range(2):
                        k0 = r0 // 2 + (2 * c + jp) * 64
                        out[sub, :, k0 : k0 + 64, :] = rows[:, :, c, jp, :]
    return [out[i].reshape(ys.shape[0], C, H // 2, W // 2) for i in range(4)]


def kernel(x):
    return _run(x, trace=False)
